# revision 27
# baseline (speedup 1.0000x reference)
"""Trainium2 Bass kernel for nn_FEFM_35218731827351.

Math (validated vs reference in fp64):
  With D the 4x4 unnormalized DCT-II matrix, M = D^T D = 8*I + 2*ones(4,4).
  All DCTs cancel:
    B  = per-patch-rows M-apply of F_K  (left apply)   = 8*F_K + 2*colsum
    A  = per-patch-cols M-apply of F_Q  (right apply)  = Q @ blockdiag(M)
    attn = softmax(temp * <A_i, B_j>_pixels)           [c, c]  (near one-hot!)
    G  = per-patch (4x4) product F_Q_p @ B_p
    Fcfr = attn @ G                                    (channel mix)
    out  = Fcfr @ (F_Q - 0.5*F_V) + F_V                (per-channel HxW matmul)

Precision: fp16 matmuls run at full bf16 rate on the TRN2 PE and carry 11
mantissa bits, enough for the near-one-hot softmax logits; everything
downstream of softmax only needs ~1e-2.  All convs and the Gram run as
single fp16 streams with fp32 PSUM accumulation.

Structure: one software pipeline over 16 8-row stripes (conv matmuls ->
evictions -> xbar transposes -> Bm-apply -> partial Gram -> per-patch
blockmul).  The blockmul materializes the Q-side broadcast via ACT copies
so the DVE multiply/accumulate chain runs in its 2x packed mode.  T =
F_Q-0.5*F_V and F_V are written straight into SBUF in [c, w, h] layout;
the tail xbar-transposes them (chunked by channel) into the [h, w, c]
layout the final per-channel matmuls need -- no DRAM round trip.  The
channel mix is computed pre-transposed on the PE (lhsT = G row-block,
rhs = attn^T), so Fcfr lands directly in [w, h, c] layout.  The final
+F_V is a mixed-dtype DVE add from PSUM.

Sharding: data-parallel over batch B=8 across the 8 NeuronCores (one batch
element per core); conv weights / temperature replicated.
"""

import numpy as np

C, H, W = 96, 128, 128
NCORES = 8


# ---------------------------------------------------------------------------
# Workaround: walrus CoreV3 setupSyncWait rejects multi-wait instructions in
# this environment; TileContext's exit drain accumulates one wait per busy
# proc.  Split them one-per-nop.
def _patch_tile_drain(tile, mybir):
    from concourse.vector_clock import ScopedClock

    if getattr(tile.TileContext, "_fefm_drain_patched", False):
        return

    def _patched(self, tick_clock, wait_clock):
        nc = self.nc
        drain_inst = nc.sync.drain(fusable=False)
        wait_clock.add_sem_waits(
            drain_inst.ins, ScopedClock({None: tick_clock.global_clock})
        )
        si = drain_inst.ins.sync_info
        if si is not None and si.on_wait and len(si.on_wait) > 1:
            waits = list(si.on_wait)
            drain_inst.ins.sync_info = mybir.SyncInfo(
                on_wait=[waits[0]], on_update=list(si.on_update)
            )
            for w in waits[1:]:
                ni = nc.sync.nop(nofuse=True, hint="split_drain_wait")
                ni.ins.sync_info = mybir.SyncInfo(on_wait=[w], on_update=[])

        nc.all_engine_barrier()
        assert self.sems is not None
        popped = nc._tile_sem_poison_stack.pop()
        assert popped is self._sem_poison
        nc.clear_and_free_semaphores(list(self.sems.allocated().values()))
        nc.all_engine_barrier()

    tile.TileContext._drain_and_barrier = _patched
    tile.TileContext._fefm_drain_patched = True


def _split_multi_waits(nc, mybir, max_waits=1):
    """Walrus CoreV2/V3 setupSyncWait rejects instructions carrying more
    than `max_waits` semaphore waits.  Hoist the excess onto engine-matched
    NoOps inserted immediately before the offending instruction (same queue,
    same program order => identical blocking semantics)."""
    n_split = 0
    for fn in nc.m.functions:
        for bb in fn.blocks:
            insts = list(bb.instructions)
            newlist = []
            changed = False
            for ins in insts:
                si = ins.sync_info
                if si is not None and si.on_wait and len(si.on_wait) > max_waits:
                    waits = list(si.on_wait)
                    extra, keep = waits[:-max_waits], waits[-max_waits:]
                    for k in range(0, len(extra), max_waits):
                        nop = mybir.InstNoOp(
                            name=f"{ins.name}-ws{k}",
                            engine=ins.engine,
                            text_hint="wait_split",
                            bass_nofuse=True,
                        )
                        nop.sync_info = mybir.SyncInfo(
                            on_wait=extra[k:k + max_waits], on_update=[])
                        newlist.append(nop)
                        n_split += 1
                    ins.sync_info = mybir.SyncInfo(
                        on_wait=keep, on_update=list(si.on_update))
                    changed = True
                newlist.append(ins)
            if changed:
                bb.instructions = newlist
    return n_split


def build_bass(split_waits=True, debug=False):
    import concourse.bass as bass
    import concourse.tile as tile
    from concourse import mybir
    from contextlib import ExitStack

    _patch_tile_drain(tile, mybir)

    dt = mybir.dt
    AF = mybir.ActivationFunctionType
    ALU = mybir.AluOpType
    X = mybir.AxisListType.X

    def bcast(ap, pos, count):
        newap = [list(d) for d in ap.ap]
        newap.insert(pos, [0, count])
        return bass.AP(tensor=ap.tensor, offset=ap.offset, ap=newap)

    def col_bcast(col_ap, n):
        return bass.AP(tensor=col_ap.tensor, offset=col_ap.offset,
                       ap=[list(col_ap.ap[0]), [0, n]])

    nc = bass.Bass()
    # padded (130x130) fp16 inputs
    x1d = nc.declare_dram_parameter("x1", [C, H + 2, W + 2], dt.float16,
                                    isOutput=False)
    x2d = nc.declare_dram_parameter("x2", [C, H + 2, W + 2], dt.float16,
                                    isOutput=False)
    # conv weights [cin, tap, cout] fp16
    wqd = nc.declare_dram_parameter("wq16", [C, 9, C], dt.float16, isOutput=False)
    wkd = nc.declare_dram_parameter("wk16", [C, 9, C], dt.float16, isOutput=False)
    wvd = nc.declare_dram_parameter("wv16", [C, 9, C], dt.float16, isOutput=False)
    bqd = nc.declare_dram_parameter("bq", [C, 1], dt.float32, isOutput=False)
    bvd = nc.declare_dram_parameter("bv", [C, 1], dt.float32, isOutput=False)
    bk16d = nc.declare_dram_parameter("bk16", [C, 1], dt.float32, isOutput=False)
    tcold = nc.declare_dram_parameter("tcol", [C, 1], dt.float32, isOutput=False)
    ntcold = nc.declare_dram_parameter("ntcol", [C, 1], dt.float32, isOutput=False)
    bmd = nc.declare_dram_parameter("bm", [128, 128], dt.float16, isOutput=False)
    identd = nc.declare_dram_parameter("ident", [C, C], dt.float16, isOutput=False)
    out_d = nc.declare_dram_parameter("out", [C, H, W], dt.float32, isOutput=True)
    if debug:
        dbg = {
            "dbg_G": nc.declare_dram_parameter("dbg_G", [C, H * W], dt.float16, isOutput=True),
            "dbg_attnT": nc.declare_dram_parameter("dbg_attnT", [C, C], dt.float16, isOutput=True),
            "dbg_Thcw": nc.declare_dram_parameter("dbg_Thcw", [128, C, W], dt.float16, isOutput=True),
            "dbg_Vhcw": nc.declare_dram_parameter("dbg_Vhcw", [128, C, W], dt.float16, isOutput=True),
            "dbg_FcT": nc.declare_dram_parameter("dbg_FcT", [128, H, C], dt.float16, isOutput=True),
        }

    # DRAM staging for the final phase in [h, c, w] layout: the conv-phase
    # stores do the partition swap (descriptor-heavy but fully overlapped),
    # the tail prefetch is a few large contiguous loads.
    dT = nc.dram_tensor("dT", [H, C, W], dt.float16)
    dFV = nc.dram_tensor("dFV", [H, C, W], dt.float16)

    with tile.TileContext(nc) as tc, ExitStack() as top:

        def pool_open(name, bufs=1, space="SBUF"):
            es = ExitStack()
            p = es.enter_context(tc.tile_pool(name=name, bufs=bufs, space=space))
            top.push(es)
            return es, p

        _, singles = pool_open("singles")
        wq_sb = singles.tile([C, 9, C], dt.float16)
        wk_sb = singles.tile([C, 9, C], dt.float16)
        wv_sb = singles.tile([C, 9, C], dt.float16)
        bq_sb = singles.tile([C, 1], dt.float32)
        bv_sb = singles.tile([C, 1], dt.float32)
        bk16_sb = singles.tile([C, 1], dt.float32)
        tcol_sb = singles.tile([C, 1], dt.float32)
        ntcol_sb = singles.tile([C, 1], dt.float32)
        bm_sb = singles.tile([128, 128], dt.float16)
        ident_sb = singles.tile([C, C], dt.float16)
        attnT = singles.tile([C, C], dt.float16)
        G_sb = singles.tile([C, H * W], dt.float16)      # blockmul out
        FcT = singles.tile([128, H, C], dt.float16)      # Fcfr in [w, h, c]
        # weights split across both issue queues so the first conv starts fast
        for w_sb, w_d in ((wq_sb, wqd), (wk_sb, wkd), (wv_sb, wvd)):
            nc.sync.dma_start(out=w_sb[:, 0:5, :], in_=w_d[:, 0:5, :])
            nc.scalar.dma_start(out=w_sb[:, 5:9, :], in_=w_d[:, 5:9, :])
        for i, (t, d) in enumerate((
            (bq_sb, bqd), (bv_sb, bvd), (bk16_sb, bk16d),
            (tcol_sb, tcold), (ntcol_sb, ntcold),
            (bm_sb, bmd), (ident_sb, identd),
        )):
            (nc.sync if i % 2 == 0 else nc.scalar).dma_start(out=t, in_=d[:])

        # ------------------------------------------------------------------
        # Stripe pipeline.
        pipe_es, pipe = pool_open("pipe", bufs=2)
        pipe3_es, pipe3 = pool_open("pipe3", bufs=3)
        ps_es, psp = pool_open("pipe_ps", bufs=2, space="PSUM")
        ps1_es, psp1 = pool_open("pipe_ps1", bufs=1, space="PSUM")
        gram_es, gram_pool = pool_open("gram_ps", bufs=1, space="PSUM")
        gram_ps = gram_pool.tile([C, C], dt.float32, tag="gram")

        # Three pipeline stages, emitted 2 stripes apart so the in-order PE
        # queue never waits on DMA/DVE/ACT latency:
        #   conv(s) || xbar+Bm-apply(s-1) || Gram+blockmul(s-2)
        fld = {}   # per-stripe field tiles carried between stages
        xpose = {}
        loaded = {}

        def load_stage(s):
            r0 = s * 8
            x1s = pipe.tile([C, 10, 130], dt.float16, tag="x1s")
            x2s = pipe.tile([C, 10, 130], dt.float16, tag="x2s")
            nc.sync.dma_start(out=x1s, in_=x1d[:, r0:r0 + 10, :])
            nc.scalar.dma_start(out=x2s, in_=x2d[:, r0:r0 + 10, :])
            loaded[s] = (x1s, x2s)

        def conv_stage(s):
            r0 = s * 8
            x1s, x2s = loaded.pop(s)

            # per-stripe field tiles [c, 8 rows, 128]; read up to 2 stages
            # later -> bufs=3 on these tags
            fq = pipe3.tile([C, 8, 128], dt.float16, tag="fq")
            bf = pipe3.tile([C, 8, 128], dt.float16, tag="bf")
            fld[s] = (fq, bf)

            # V conv: single-buffered 8-row stream (PSUM bank pressure is
            # lower and the eviction happens once per stripe)
            psV = psp1.tile([C, 8, 128], dt.float32, tag="psV")
            for chunk in range(2):
                y0 = r0 + chunk * 4
                c0 = chunk * 4
                psQ = psp.tile([C, 4, 128], dt.float32, tag="psQ")
                psK = psp.tile([C, 4, 128], dt.float32, tag="psK")
                for t9 in range(9):
                    dy, dx = divmod(t9, 3)
                    rsl = (slice(None), slice(c0 + dy, c0 + dy + 4),
                           slice(dx, dx + 128))
                    r1 = x1s[rsl]
                    st, sp = t9 == 0, t9 == 8
                    nc.tensor.matmul(psQ, lhsT=wq_sb[:, t9, :], rhs=r1,
                                     start=st, stop=sp)
                    nc.tensor.matmul(psK, lhsT=wk_sb[:, t9, :], rhs=r1,
                                     start=st, stop=sp)
                    # psV slice stays inside one PSUM bank per group
                    nc.tensor.matmul(psV[:, c0:c0 + 4, :],
                                     lhsT=wv_sb[:, t9, :], rhs=x2s[rsl],
                                     start=st, stop=sp)

                csl = slice(c0, c0 + 4)
                # F_Q chunk (+bias)
                nc.scalar.activation(fq[:, csl, :], psQ, AF.Identity,
                                     bias=bq_sb[:])
                # B chunk = 8*psK + 2*colsum(psK) + 16*bk
                s_t = pipe.tile([C, 128], dt.float32, tag="scol")
                s2_t = pipe.tile([C, 128], dt.float32, tag="scol2")
                nc.vector.tensor_reduce(
                    out=s_t, in_=psK[:].rearrange("c h w -> c w h"),
                    axis=X, op=ALU.add)
                nc.vector.scalar_tensor_tensor(
                    out=s2_t, in0=s_t, scalar=2.0,
                    in1=col_bcast(bk16_sb[:], 128),
                    op0=ALU.mult, op1=ALU.add)
                nc.vector.scalar_tensor_tensor(
                    out=bf[:, csl, :], in0=psK, scalar=8.0,
                    in1=bcast(s2_t[:], 1, 4),
                    op0=ALU.mult, op1=ALU.add)

            # F_V stripe (+bias) -> DRAM; T = F_Q - 0.5*F_V -> DRAM, both
            # stored in [h, c, w] layout for the tail prefetch
            vstg = pipe.tile([C, 8, 128], dt.float16, tag="vstg")
            tstg = pipe.tile([C, 8, 128], dt.float16, tag="tstg")
            nc.scalar.activation(vstg, psV, AF.Identity, bias=bv_sb[:])
            nc.sync.dma_start(
                out=dFV[r0:r0 + 8].rearrange("y c w -> c y w"), in_=vstg)
            nc.vector.scalar_tensor_tensor(
                out=tstg, in0=vstg, scalar=-0.5, in1=fq,
                op0=ALU.mult, op1=ALU.add)
            nc.sync.dma_start(
                out=dT[r0:r0 + 8].rearrange("y c w -> c y w"), in_=tstg)

        mid_t = {}

        def mid_xbar(s):
            # xbar transposes of stripe s's fields to [w, h8, c]; issued at
            # iteration start so the sync queue has a full stripe of slack
            fq, bf = fld[s]
            Qc = pipe.tile([128, 8, C], dt.float16, tag="Qc")
            Bc = pipe3.tile([128, 8, C], dt.float16, tag="Bc")
            nc.sync.dma_start_transpose(out=Qc, in_=fq[:].rearrange("c a b -> c (a b)"))
            nc.sync.dma_start_transpose(out=Bc, in_=bf[:].rearrange("c a b -> c (a b)"))
            mid_t[s] = (Qc, Bc)

        def mid_mm(s):
            # A_T = Bm @ Q_T on the PE
            Qc, Bc = mid_t.pop(s)
            A2 = pipe3.tile([128, 8, C], dt.float16, tag="A2")
            for sub in range(2):
                ssl = slice(sub * 4, sub * 4 + 4)
                ps = psp1.tile([128, 4, C], dt.float32, tag="at")
                nc.tensor.matmul(ps, lhsT=bm_sb[:], rhs=Qc[:, ssl, :],
                                 start=True, stop=True)
                nc.scalar.activation(A2[:, ssl, :], ps, AF.Copy)
            xpose[s] = (A2, Bc)

        def tail_stage(s):
            # partial Gram over stripe s's 8 h-rows
            A2, Bc = xpose.pop(s)
            for h in range(8):
                first = s == 0 and h == 0
                last = s == 15 and h == 7
                nc.tensor.matmul(gram_ps, lhsT=A2[:, h, :], rhs=Bc[:, h, :],
                                 start=first, stop=last)

        def blockmul_stage(s):
            # blockmul for stripe s's 2 patch-row groups, written directly
            # into the SBUF G tile.  The Q-side d-broadcast is materialized
            # by ACT copies so every DVE operand is innermost-packed fp16
            # (2x_1p mode).
            fq, bf = fld.pop(s)
            Q5 = fq[:].rearrange("c (i a) (j d) -> c i a j d", a=4, d=4)
            B5 = bf[:].rearrange("c (i a) (j d) -> c i a j d", a=4, d=4)
            Gq = G_sb[:, s * 1024:(s + 1) * 1024].rearrange(
                "c (i a j d) -> c i a j d", i=2, a=4, j=32)
            for b4 in range(4):
                Qb = pipe.tile([C, 2, 4, 32, 4], dt.float16, tag="qb")
                nc.scalar.activation(Qb, bcast(Q5[:, :, :, :, b4], 4, 4),
                                     AF.Copy)
                bvv = bcast(B5[:, :, b4, :, :], 2, 4)   # bcast a, inner packed
                if b4 == 0:
                    nc.vector.tensor_tensor(out=Gq, in0=Qb, in1=bvv,
                                            op=ALU.mult)
                else:
                    Pt = pipe.tile([C, 2, 4, 32, 4], dt.float16, tag="pv")
                    nc.vector.tensor_tensor(out=Pt, in0=Qb, in1=bvv,
                                            op=ALU.mult)
                    nc.vector.tensor_add(Gq, Gq, Pt)

        load_stage(0)
        for s in range(18):
            if s + 1 < 16:
                load_stage(s + 1)
            if 1 <= s <= 16:
                mid_xbar(s - 1)
            if 2 <= s:
                tail_stage(s - 2)
            if s < 16:
                conv_stage(s)
            if 1 <= s <= 16:
                mid_mm(s - 1)
            if 2 <= s:
                blockmul_stage(s - 2)

        # ---- softmax + attn transpose (uses gram_ps, inside pipe psum scope)
        sm_es, sm_pool = pool_open("softmax")
        m_sb = sm_pool.tile([C, 1], dt.float32)
        negtm = sm_pool.tile([C, 1], dt.float32)
        z_sb = sm_pool.tile([C, C], dt.float32)
        e_sb = sm_pool.tile([C, C], dt.float32)
        rs_sb = sm_pool.tile([C, 1], dt.float32)
        r_sb = sm_pool.tile([C, 1], dt.float32)
        attn_n = sm_pool.tile([C, C], dt.float16)
        nc.vector.tensor_reduce(out=m_sb, in_=gram_ps[:], axis=X, op=ALU.max)
        nc.vector.tensor_mul(negtm, m_sb, ntcol_sb)
        # z = temp*logit - temp*max, clamped (HW Exp misbehaves far below 0)
        nc.vector.scalar_tensor_tensor(out=z_sb, in0=gram_ps[:],
                                       scalar=tcol_sb[:, 0:1],
                                       in1=col_bcast(negtm[:], C),
                                       op0=ALU.mult, op1=ALU.add)
        nc.vector.tensor_scalar_max(z_sb, z_sb, -60.0)
        nc.scalar.activation(e_sb, z_sb, AF.Exp, accum_out=rs_sb[:])
        nc.vector.reciprocal(r_sb, rs_sb)
        nc.scalar.activation(attn_n, e_sb, AF.Copy, scale=r_sb[:])
        psT = psp1.tile([C, C], dt.float16, tag="at")
        nc.tensor.transpose(psT, attn_n, ident_sb[:])
        nc.scalar.activation(attnT, psT, AF.Copy)
        sm_es.close()
        gram_es.close()
        ps1_es.close()
        ps_es.close()
        pipe3_es.close()
        pipe_es.close()

        # ------------------------------------------------------------------
        # Prefetch T/V into SBUF with large contiguous loads spread over
        # both issue queues (the [h, c, w] DRAM layout makes each 1KB+
        # contiguous per descriptor); overlaps the softmax + mix phases.
        tv_es, tv_pool = pool_open("tv")
        T_hcw = tv_pool.tile([128, C, W], dt.float16)
        V_hcw = tv_pool.tile([128, C, W], dt.float16)
        for ci, cc in enumerate(range(0, C, 12)):
            eng = nc.sync if ci % 2 == 0 else nc.scalar
            eng.dma_start(out=T_hcw[:, cc:cc + 12, :], in_=dT[:, cc:cc + 12, :])
            eng2 = nc.scalar if ci % 2 == 0 else nc.sync
            eng2.dma_start(out=V_hcw[:, cc:cc + 12, :], in_=dFV[:, cc:cc + 12, :])

        # ------------------------------------------------------------------
        # Channel mix computed pre-transposed: FcT[:, h, :] = G_row_h^T @
        # attn^T, so Fcfr lands directly in [w, h, c] layout on the PE.
        mix_ps_es, mix_ps = pool_open("mix_ps", bufs=4, space="PSUM")
        for g in range(32):
            ps = mix_ps.tile([128, 4, C], dt.float32, tag="mix")
            for hh in range(4):
                h = g * 4 + hh
                nc.tensor.matmul(ps[:, hh, :],
                                 lhsT=G_sb[:, h * 128:(h + 1) * 128],
                                 rhs=attnT[:], start=True, stop=True)
            nc.scalar.activation(FcT[:, g * 4:g * 4 + 4, :], ps, AF.Copy)
        mix_ps_es.close()

        if debug:
            nc.sync.dma_start(out=dbg["dbg_G"][:], in_=G_sb)
            nc.sync.dma_start(out=dbg["dbg_attnT"][:], in_=attnT)
            nc.sync.dma_start(out=dbg["dbg_Thcw"][:], in_=T_hcw)
            nc.sync.dma_start(out=dbg["dbg_Vhcw"][:], in_=V_hcw)
            nc.sync.dma_start(out=dbg["dbg_FcT"][:], in_=FcT)

        # ------------------------------------------------------------------
        # Final per-channel matmuls out_c = Fcfr_c @ T_c + F_V_c; the +F_V
        # is a mixed-dtype DVE add straight from PSUM.
        outst_es, outst = pool_open("outst", bufs=8)
        fin_ps_es, fin_ps = pool_open("fin_ps", bufs=4, space="PSUM")
        for i, c0 in enumerate(range(0, C, 4)):
            ps2 = fin_ps.tile([128, 4, 128], dt.float32, tag="final")
            for c4 in range(4):
                c = c0 + c4
                nc.tensor.matmul(ps2[:, c4, :], lhsT=FcT[:, :, c],
                                 rhs=T_hcw[:, c, :], start=True, stop=True)
            ob = outst.tile([128, 4, 128], dt.float32, tag="ob")
            nc.vector.tensor_add(ob, ps2, V_hcw[:, c0:c0 + 4, :])
            eng = nc.sync if i % 2 == 0 else nc.scalar
            eng.dma_start(out=out_d[c0:c0 + 4].rearrange("c h w -> h c w"),
                          in_=ob)
        fin_ps_es.close()
        outst_es.close()
        tv_es.close()

    if split_waits:
        # Skipped for CoreSim runs -- the sim's race detector only knows
        # instructions registered through the builder API.
        _split_multi_waits(nc, mybir)
    return nc


def host_prep(input1, input2, wq, bq, wk, bk, wv, bv, temperature):
    f16 = np.float16
    f32 = np.float32

    def wprep(w):
        # [cout, cin, 3, 3] -> [cin, tap, cout] fp16
        wt = np.ascontiguousarray(
            np.transpose(np.asarray(w, f32), (1, 2, 3, 0)).reshape(C, 9, C))
        return wt.astype(f16)

    temp = float(np.asarray(temperature, f32).reshape(-1)[0])
    tcol = np.full((C, 1), temp, f32)
    M4 = 8.0 * np.eye(4) + 2.0 * np.ones((4, 4))
    bm = np.kron(np.eye(32), M4).astype(f16)
    common = {
        "wq16": wprep(wq), "wk16": wprep(wk), "wv16": wprep(wv),
        "bq": np.asarray(bq, f32).reshape(C, 1),
        "bv": np.asarray(bv, f32).reshape(C, 1),
        "bk16": 16.0 * np.asarray(bk, f32).reshape(C, 1),
        "tcol": tcol, "ntcol": -tcol,
        "bm": bm,
        "ident": np.eye(C, dtype=f16),
    }
    x1p = np.pad(np.asarray(input1, f32), ((0, 0), (0, 0), (1, 1), (1, 1)))
    x2p = np.pad(np.asarray(input2, f32), ((0, 0), (0, 0), (1, 1), (1, 1)))
    x1h = x1p.astype(f16)
    x2h = x2p.astype(f16)
    maps = []
    for i in range(NCORES):
        m = dict(common)
        m["x1"] = np.ascontiguousarray(x1h[i])
        m["x2"] = np.ascontiguousarray(x2h[i])
        maps.append(m)
    return maps


_NC = None


def kernel(input1, input2, wq, bq, wk, bk, wv, bv, temperature):
    global _NC
    from concourse.bass_utils import run_bass_kernel_spmd

    if _NC is None:
        _NC = build_bass()
    in_maps = host_prep(input1, input2, wq, bq, wk, bk, wv, bv, temperature)
    res = run_bass_kernel_spmd(_NC, in_maps, list(range(NCORES)))
    out = np.stack([np.asarray(res.results[i]["out"]) for i in range(NCORES)])
    return out.astype(np.float32)
